# revision 4
# baseline (speedup 1.0000x reference)
"""Trainium2 Bass kernel for nn_Attention_9981503996487.

Single-layer attention prefill (B=1, S=4096, H=2048, 16 q-heads, 4 kv-heads,
D=128, RoPE, causal, GQA, empty KV cache at cache_position=0).

Sharding (tensor parallel over heads, per the hint): core c owns q-heads
{2c, 2c+1} and kv-head c//2.  wq/wk/wv are split column-wise, wo row-wise;
each core computes a partial o_proj output over its 256 head-channels and
the host sums the 8 partials (the "all-reduce").

Per-core device program (all SBUF tensors fp32; matmuls run as float32r):
  A) QKV projection: hiddenT is streamed in 512-column chunks; a packed
     [2048, 512] weight block (q0|q1|k|v) accumulates 4 PSUM tiles over 16
     contraction tiles, producing qT/kT/vT in [d=128, s] layout.  RoPE is
     applied in that layout via a DMA half-swap plus sign-folded sin.  V is
     transposed to [s, d] tiles with PE transposes.
  B) Flash-style causal attention per (head, 512-query chunk): scoresT tile
     [k=128, q=512] = kT_tile^T @ qT_chunk; exp on ACT (PSUM->SBUF) with the
     1/sqrt(D) scale; multiplicative causal mask on the 4 diagonal tiles;
     O^T accumulation via V_tile^T @ P; softmax denominators via ones^T @ P;
     normalization by reciprocal broadcast over partitions with a K=1 outer
     product matmul.
  C) o_proj: out[s_tile, hid_chunk] accumulated over the two heads' OT
     slices; result DMA'd to DRAM as the core's partial output.
"""

import math
import os

import numpy as np

S = 4096
HID = 2048
NH = 16
NKV = 4
D = 128
NCORES = 8
CH = 512          # query / s-chunk width
NCH = S // CH     # 8 chunks
NT = HID // 128   # 16 contraction tiles
SCALE = 1.0 / math.sqrt(D)


def _build_nc():
    import concourse.bacc as bacc
    import concourse.mybir as mybir
    import concourse.tile as tile

    f32 = mybir.dt.float32
    f32r = mybir.dt.float32r
    EXP = mybir.ActivationFunctionType.Exp

    nc = bacc.Bacc("TRN2", target_bir_lowering=False, debug=False)

    hT = nc.dram_tensor("hT", [HID, S], f32r, kind="ExternalInput")
    wcat = nc.dram_tensor("wcat", [HID, 512], f32r, kind="ExternalInput")
    wo2 = nc.dram_tensor("wo2", [256, HID], f32r, kind="ExternalInput")
    cosT = nc.dram_tensor("cosT", [128, S], f32r, kind="ExternalInput")
    sinTs = nc.dram_tensor("sinTs", [128, S], f32r, kind="ExternalInput")
    mask4 = nc.dram_tensor("mask4", [128, 4 * CH], f32r, kind="ExternalInput")
    ident = nc.dram_tensor("ident", [128, 128], f32r, kind="ExternalInput")
    onesm = nc.dram_tensor("onesm", [128, 128], f32r, kind="ExternalInput")
    out = nc.dram_tensor("out", [S, HID], f32, kind="ExternalOutput")

    with tile.TileContext(nc) as tc:
        with tc.tile_pool(name="persist", bufs=1) as persist:
            qt0 = persist.tile([128, S], f32r, name="qt0")
            qt1 = persist.tile([128, S], f32r, name="qt1")
            ktt = persist.tile([128, S], f32r, name="ktt")
            vsb = persist.tile([128, S], f32r, name="vsb")
            id_sb = persist.tile([128, 128], f32r, name="id_sb")
            ones_sb = persist.tile([128, 128], f32r, name="ones_sb")
            nc.sync.dma_start(id_sb[:], ident[:])
            nc.sync.dma_start(ones_sb[:], onesm[:])
            qdest = [qt0, qt1, ktt]

            # ---------------- Stage A: QKV projection + RoPE ----------------
            with (
                tc.tile_pool(name="aw", bufs=1) as aw,
                tc.tile_pool(name="ah", bufs=2) as ah,
                tc.tile_pool(name="ax", bufs=3) as ax,
                tc.tile_pool(name="psA", bufs=2, space="PSUM") as psA,
                tc.tile_pool(name="psTR", bufs=2, space="PSUM") as psTR,
            ):
                wcat_sb = aw.tile([128, NT * 512], f32r, name="wcat_sb")
                for t in range(NT):
                    nc.sync.dma_start(
                        wcat_sb[:, t * 512:(t + 1) * 512],
                        wcat[t * 128:(t + 1) * 128, :],
                    )

                for ci in range(NCH):
                    s0 = ci * CH
                    htile = ah.tile([128, NT * CH], f32r, name="htile", tag="htile")
                    for t in range(NT):
                        nc.sync.dma_start(
                            htile[:, t * CH:(t + 1) * CH],
                            hT[t * 128:(t + 1) * 128, s0:s0 + CH],
                        )
                    cos_c = ah.tile([128, CH], f32r, name="cos_c", tag="cos_c")
                    nc.sync.dma_start(cos_c[:], cosT[:, s0:s0 + CH])
                    sin_c = ah.tile([128, CH], f32r, name="sin_c", tag="sin_c")
                    nc.sync.dma_start(sin_c[:], sinTs[:, s0:s0 + CH])

                    for o in range(4):
                        ps = psA.tile([128, CH], f32, name="psA_t", tag="psA_t")
                        for t in range(NT):
                            wsl = wcat_sb[:, t * 512 + o * 128:t * 512 + (o + 1) * 128]
                            nc.tensor.matmul(
                                ps[:], wsl, htile[:, t * CH:(t + 1) * CH],
                                start=(t == 0), stop=(t == NT - 1),
                            )
                        x_sb = ax.tile([128, CH], f32r, name="x_sb", tag="evac")
                        nc.scalar.copy(x_sb[:], ps[:])
                        if o < 3:
                            # RoPE: dest = x * cosT + halfswap(x) * signed_sinT
                            swap = ax.tile([128, CH], f32r, name="swap", tag="swap")
                            nc.sync.dma_start(swap[0:64, :], x_sb[64:128, :])
                            nc.sync.dma_start(swap[64:128, :], x_sb[0:64, :])
                            t1 = ax.tile([128, CH], f32r, name="t1", tag="t1")
                            nc.vector.tensor_mul(t1[:], x_sb[:], cos_c[:])
                            t2 = ax.tile([128, CH], f32r, name="t2", tag="t2")
                            nc.vector.tensor_mul(t2[:], swap[:], sin_c[:])
                            nc.vector.tensor_add(
                                qdest[o][:, s0:s0 + CH], t1[:], t2[:]
                            )
                        else:
                            # V: transpose [d, s] -> [s, d] blocks
                            trp = psTR.tile([128, CH], f32r, name="trp", tag="trp")
                            for b in range(4):
                                nc.tensor.transpose(
                                    trp[:, b * 128:(b + 1) * 128],
                                    x_sb[:, b * 128:(b + 1) * 128],
                                    id_sb[:],
                                )
                            nc.scalar.copy(vsb[:, s0:s0 + CH], trp[:])

            # ---------------- Stage B + C: attention and o_proj ----------------
            with (
                tc.tile_pool(name="bw", bufs=1) as bw,
                tc.tile_pool(name="bp", bufs=3) as bp,
                tc.tile_pool(name="bo", bufs=4) as bo,
                tc.tile_pool(name="br", bufs=2) as brp,
                tc.tile_pool(name="co", bufs=4) as co,
                tc.tile_pool(name="psST", bufs=3, space="PSUM") as psST,
                tc.tile_pool(name="psOT", bufs=2, space="PSUM") as psOT,
                tc.tile_pool(name="psDEN", bufs=1, space="PSUM") as psDEN,
                tc.tile_pool(name="psO", bufs=2, space="PSUM") as psO,
            ):
                mask_sb = bw.tile([128, 4 * CH], f32r, name="mask_sb")
                nc.sync.dma_start(mask_sb[:], mask4[:])
                wo_sb0 = bw.tile([128, HID], f32r, name="wo_sb0")
                nc.sync.dma_start(wo_sb0[:], wo2[0:128, :])
                wo_sb1 = bw.tile([128, HID], f32r, name="wo_sb1")
                nc.sync.dma_start(wo_sb1[:], wo2[128:256, :])

                for ci in range(NCH):
                    s0 = ci * CH
                    ot_tiles = []
                    for h in range(2):
                        qt = [qt0, qt1][h]
                        n_kt = 4 * (ci + 1)
                        ot_ps = psOT.tile([128, CH], f32, name="ot_ps", tag="ot")
                        den_ps = psDEN.tile([1, CH], f32, name="den_ps", tag="den")
                        for kt in range(n_kt):
                            st_ps = psST.tile([128, CH], f32, name="st_ps", tag="st")
                            nc.tensor.matmul(
                                st_ps[:],
                                ktt[:, kt * 128:(kt + 1) * 128],
                                qt[:, s0:s0 + CH],
                                start=True, stop=True,
                            )
                            p_sb = bp.tile([128, CH], f32r, name="p_sb", tag="p")
                            nc.scalar.activation(p_sb[:], st_ps[:], EXP, scale=SCALE)
                            if kt >= 4 * ci:
                                ridx = kt - 4 * ci
                                nc.vector.tensor_mul(
                                    p_sb[:], p_sb[:],
                                    mask_sb[:, ridx * CH:(ridx + 1) * CH],
                                )
                            nc.tensor.matmul(
                                ot_ps[:],
                                vsb[:, kt * 128:(kt + 1) * 128],
                                p_sb[:],
                                start=(kt == 0), stop=(kt == n_kt - 1),
                            )
                            nc.tensor.matmul(
                                den_ps[:],
                                ones_sb[:, 0:1],
                                p_sb[:],
                                start=(kt == 0), stop=(kt == n_kt - 1),
                            )
                        recip = brp.tile([1, CH], f32r, name="recip", tag="recip")
                        with nc.allow_low_precision(reason="f32r softmax denom"):
                            nc.vector.reciprocal(recip[:], den_ps[:])
                        bc_ps = psST.tile([128, CH], f32, name="bc_ps", tag="st")
                        nc.tensor.matmul(
                            bc_ps[:], ones_sb[0:1, :], recip[:],
                            start=True, stop=True,
                        )
                        bc_sb = brp.tile([128, CH], f32r, name="bc_sb", tag="bc_sb")
                        nc.scalar.copy(bc_sb[:], bc_ps[:])
                        ot_sb = bo.tile([128, CH], f32r, name="ot_sb", tag=f"ot{h}")
                        nc.vector.tensor_mul(ot_sb[:], ot_ps[:], bc_sb[:])
                        ot_tiles.append(ot_sb)

                    # Stage C: o_proj for this chunk's 4 row-tiles
                    for st_i in range(4):
                        row = (ci * 4 + st_i) * 128
                        for hc in range(4):
                            ops = psO.tile([128, 512], f32, name="ops", tag="ops")
                            nc.tensor.matmul(
                                ops[:],
                                ot_tiles[0][:, st_i * 128:(st_i + 1) * 128],
                                wo_sb0[:, hc * 512:(hc + 1) * 512],
                                start=True, stop=False,
                            )
                            nc.tensor.matmul(
                                ops[:],
                                ot_tiles[1][:, st_i * 128:(st_i + 1) * 128],
                                wo_sb1[:, hc * 512:(hc + 1) * 512],
                                start=False, stop=True,
                            )
                            o_sb = co.tile([128, 512], f32, name="o_sb", tag="o_sb")
                            if (st_i + hc) % 2 == 0:
                                nc.scalar.copy(o_sb[:], ops[:])
                            else:
                                nc.vector.tensor_copy(o_sb[:], ops[:])
                            nc.sync.dma_start(
                                out[row:row + 128, hc * 512:(hc + 1) * 512],
                                o_sb[:],
                            )

    nc.finalize()
    return nc


def _host_prep(hidden_states, cos, sin, position_ids, wq, wk, wv, wo):
    """Build the 8 per-core input maps (all float32 numpy arrays)."""
    hidden = np.asarray(hidden_states, dtype=np.float32)[0]        # [S, HID]
    hT = np.ascontiguousarray(hidden.T)                            # [HID, S]
    pos = np.asarray(position_ids)[0].astype(np.int64)             # [S]
    cos_np = np.asarray(cos, dtype=np.float32)[pos]                # [S, 64]
    sin_np = np.asarray(sin, dtype=np.float32)[pos]
    cos_full = np.concatenate([cos_np, cos_np], axis=1)            # [S, 128]
    sin_full = np.concatenate([sin_np, sin_np], axis=1)
    cosT = np.ascontiguousarray(cos_full.T)                        # [128, S]
    sinTs = np.ascontiguousarray(sin_full.T)
    sinTs[0:64, :] *= -1.0                                         # sign fold

    # multiplicative causal masks for the 4 diagonal tile offsets
    kk = np.arange(128)[:, None]
    jj = np.arange(CH)[None, :]
    mask4 = np.concatenate(
        [(kk + ridx * 128 <= jj).astype(np.float32) for ridx in range(4)], axis=1
    )                                                              # [128, 2048]
    ident = np.eye(128, dtype=np.float32)
    onesm = np.ones((128, 128), dtype=np.float32)

    wq_np = np.asarray(wq, dtype=np.float32)
    wk_np = np.asarray(wk, dtype=np.float32)
    wv_np = np.asarray(wv, dtype=np.float32)
    wo_np = np.asarray(wo, dtype=np.float32)

    in_maps = []
    for c in range(NCORES):
        h0 = 2 * c
        g = c // 2
        wcat = np.ascontiguousarray(np.concatenate(
            [
                wq_np[:, h0 * D:(h0 + 1) * D],
                wq_np[:, (h0 + 1) * D:(h0 + 2) * D],
                wk_np[:, g * D:(g + 1) * D],
                wv_np[:, g * D:(g + 1) * D],
            ],
            axis=1,
        ))                                                         # [HID, 512]
        wo2 = np.ascontiguousarray(wo_np[h0 * D:(h0 + 2) * D, :])  # [256, HID]
        in_maps.append({
            "hT": hT,
            "wcat": wcat,
            "wo2": wo2,
            "cosT": cosT,
            "sinTs": sinTs,
            "mask4": mask4,
            "ident": ident,
            "onesm": onesm,
        })
    return in_maps


_NC_CACHE = [None]


def _run(inputs, trace=False, tmpdir=None):
    from concourse import bass_utils

    in_maps = _host_prep(
        inputs["hidden_states"], inputs["cos"], inputs["sin"],
        inputs["position_ids"], inputs["wq"], inputs["wk"], inputs["wv"],
        inputs["wo"],
    )
    if _NC_CACHE[0] is None:
        _NC_CACHE[0] = _build_nc()
    nc = _NC_CACHE[0]
    res = bass_utils.run_bass_kernel_spmd(
        nc, in_maps, core_ids=list(range(NCORES)), trace=trace, tmpdir=tmpdir,
    )
    acc = res.results[0]["out"].astype(np.float32)
    for c in range(1, NCORES):
        acc = acc + res.results[c]["out"]
    return acc.reshape(1, S, HID), res


def kernel(**inputs):
    out, _ = _run(inputs, trace=False)
    return out


# revision 5
# speedup vs baseline: 1.0673x; 1.0673x over previous
"""Trainium2 Bass kernel for nn_Attention_9981503996487.

Single-layer attention prefill (B=1, S=4096, H=2048, 16 q-heads, 4 kv-heads,
D=128, RoPE, causal, GQA, empty KV cache at cache_position=0).

Sharding (tensor parallel over heads, per the hint): core c owns q-heads
{2c, 2c+1} and kv-head c//2.  wq/wk/wv are split column-wise, wo row-wise;
each core computes a partial o_proj output over its 256 head-channels and
the host sums the 8 partials (the "all-reduce").

Per-core device program (matmul datapath in DT = bf16 by default, fp32 PSUM):
  A) QKV projection: hiddenT streamed in 512-column chunks; a packed
     [2048, 512] weight block (q0|q1|k|v) accumulates 4 PSUM tiles over 16
     contraction tiles, producing qT/kT/vT in [d=128, s] layout.  RoPE is
     applied in that layout via a DMA half-swap plus sign-folded sin.  V is
     transposed to [s, d] tiles with PE transposes.
  B) Flash-style causal attention per (head, 512-query chunk): scoresT tile
     [k=128, q<=512] = kT_tile^T @ qT_chunk (diagonal tiles restrict the
     query-column range to the causally visible part); exp on ACT
     (PSUM->SBUF) with the 1/sqrt(D) scale; multiplicative causal mask on
     the 4 diagonal tiles; O^T accumulation via V_tile^T @ P; softmax
     denominators via ones^T @ P; normalization via an fp32 K=1 outer
     product broadcast of the denominators, a [128,512] reciprocal, and one
     DVE multiply.
  C) o_proj: out[s_tile, hid_chunk] accumulated over the two heads' OT
     slices; result DMA'd to DRAM as the core's partial fp32 output.
"""

import math
import os

import numpy as np

S = 4096
HID = 2048
D = 128
NCORES = 8
CH = 512          # query / s-chunk width
NCH = S // CH     # 8 chunks
NT = HID // 128   # 16 contraction tiles
SCALE = 1.0 / math.sqrt(D)
DT_NAME = os.environ.get("BASSK_DTYPE", "bf16")


def _build_nc():
    import concourse.bacc as bacc
    import concourse.mybir as mybir
    import concourse.tile as tile

    f32 = mybir.dt.float32
    DT = mybir.dt.bfloat16 if DT_NAME == "bf16" else mybir.dt.float32r
    # dtype for the V-transpose path (PE transpose requires out == lhsT dtype;
    # PSUM stays 4-byte)
    TDT = f32 if DT_NAME == "bf16" else mybir.dt.float32r
    EXP = mybir.ActivationFunctionType.Exp

    nc = bacc.Bacc("TRN2", target_bir_lowering=False, debug=False)

    hT = nc.dram_tensor("hT", [HID, S], DT, kind="ExternalInput")
    wcat = nc.dram_tensor("wcat", [HID, 512], DT, kind="ExternalInput")
    wo2 = nc.dram_tensor("wo2", [256, HID], DT, kind="ExternalInput")
    cosT = nc.dram_tensor("cosT", [128, S], DT, kind="ExternalInput")
    sinTs = nc.dram_tensor("sinTs", [128, S], DT, kind="ExternalInput")
    mask4 = nc.dram_tensor("mask4", [128, 4 * CH], DT, kind="ExternalInput")
    ident = nc.dram_tensor("ident", [128, 128], TDT, kind="ExternalInput")
    onesm = nc.dram_tensor("onesm", [128, 128], DT, kind="ExternalInput")
    ones32 = nc.dram_tensor("ones32", [1, 128], f32, kind="ExternalInput")
    out = nc.dram_tensor("out", [S, HID], f32, kind="ExternalOutput")

    with tile.TileContext(nc) as tc:
        with tc.tile_pool(name="persist", bufs=1) as persist:
            qt0 = persist.tile([128, S], DT, name="qt0")
            qt1 = persist.tile([128, S], DT, name="qt1")
            ktt = persist.tile([128, S], DT, name="ktt")
            vsb = persist.tile([128, S], DT, name="vsb")
            id_sb = persist.tile([128, 128], TDT, name="id_sb")
            ones_sb = persist.tile([128, 128], DT, name="ones_sb")
            ones32_sb = persist.tile([1, 128], f32, name="ones32_sb")
            nc.sync.dma_start(id_sb[:], ident[:])
            nc.sync.dma_start(ones_sb[:], onesm[:])
            nc.sync.dma_start(ones32_sb[:], ones32[:])
            qdest = [qt0, qt1, ktt]

            # ---------------- Stage A: QKV projection + RoPE ----------------
            with (
                tc.tile_pool(name="aw", bufs=1) as aw,
                tc.tile_pool(name="ah", bufs=2) as ah,
                tc.tile_pool(name="ax", bufs=3) as ax,
                tc.tile_pool(name="psA", bufs=2, space="PSUM") as psA,
                tc.tile_pool(name="psTR", bufs=2, space="PSUM") as psTR,
            ):
                wcat_sb = aw.tile([128, NT * 512], DT, name="wcat_sb")
                for t in range(NT):
                    nc.sync.dma_start(
                        wcat_sb[:, t * 512:(t + 1) * 512],
                        wcat[t * 128:(t + 1) * 128, :],
                    )

                for ci in range(NCH):
                    s0 = ci * CH
                    htile = ah.tile([128, NT * CH], DT, name="htile", tag="htile")
                    for t in range(NT):
                        nc.sync.dma_start(
                            htile[:, t * CH:(t + 1) * CH],
                            hT[t * 128:(t + 1) * 128, s0:s0 + CH],
                        )
                    cos_c = ah.tile([128, CH], DT, name="cos_c", tag="cos_c")
                    nc.sync.dma_start(cos_c[:], cosT[:, s0:s0 + CH])
                    sin_c = ah.tile([128, CH], DT, name="sin_c", tag="sin_c")
                    nc.sync.dma_start(sin_c[:], sinTs[:, s0:s0 + CH])

                    for o in range(4):
                        ps = psA.tile([128, CH], f32, name="psA_t", tag="psA_t")
                        for t in range(NT):
                            wsl = wcat_sb[:, t * 512 + o * 128:t * 512 + (o + 1) * 128]
                            nc.tensor.matmul(
                                ps[:], wsl, htile[:, t * CH:(t + 1) * CH],
                                start=(t == 0), stop=(t == NT - 1),
                            )
                        if o < 3:
                            # RoPE: dest = x * cosT + halfswap(x) * signed_sinT
                            x_sb = ax.tile([128, CH], DT, name="x_sb", tag="evac")
                            nc.scalar.copy(x_sb[:], ps[:])
                            swap = ax.tile([128, CH], DT, name="swap", tag="swap")
                            nc.sync.dma_start(swap[0:64, :], x_sb[64:128, :])
                            nc.sync.dma_start(swap[64:128, :], x_sb[0:64, :])
                            t1 = ax.tile([128, CH], DT, name="t1", tag="t1")
                            nc.vector.tensor_mul(t1[:], x_sb[:], cos_c[:])
                            t2 = ax.tile([128, CH], DT, name="t2", tag="t2")
                            nc.vector.tensor_mul(t2[:], swap[:], sin_c[:])
                            nc.vector.tensor_add(
                                qdest[o][:, s0:s0 + CH], t1[:], t2[:]
                            )
                        else:
                            # V: transpose [d, s] -> [s, d] blocks
                            x_v = ax.tile([128, CH], TDT, name="x_v", tag="evacv")
                            nc.scalar.copy(x_v[:], ps[:])
                            trp = psTR.tile([128, CH], TDT, name="trp", tag="trp")
                            for b in range(4):
                                nc.tensor.transpose(
                                    trp[:, b * 128:(b + 1) * 128],
                                    x_v[:, b * 128:(b + 1) * 128],
                                    id_sb[:],
                                )
                            nc.scalar.copy(vsb[:, s0:s0 + CH], trp[:])

            # ---------------- Stage B + C: attention and o_proj ----------------
            with (
                tc.tile_pool(name="bw", bufs=1) as bw,
                tc.tile_pool(name="bp", bufs=3) as bp,
                tc.tile_pool(name="bo", bufs=4) as bo,
                tc.tile_pool(name="br", bufs=2) as brp,
                tc.tile_pool(name="co", bufs=4) as co,
                tc.tile_pool(name="psST", bufs=3, space="PSUM") as psST,
                tc.tile_pool(name="psOT", bufs=2, space="PSUM") as psOT,
                tc.tile_pool(name="psDEN", bufs=1, space="PSUM") as psDEN,
                tc.tile_pool(name="psO", bufs=2, space="PSUM") as psO,
            ):
                mask_sb = bw.tile([128, 4 * CH], DT, name="mask_sb")
                nc.sync.dma_start(mask_sb[:], mask4[:])
                wo_sb0 = bw.tile([128, HID], DT, name="wo_sb0")
                nc.sync.dma_start(wo_sb0[:], wo2[0:128, :])
                wo_sb1 = bw.tile([128, HID], DT, name="wo_sb1")
                nc.sync.dma_start(wo_sb1[:], wo2[128:256, :])

                for ci in range(NCH):
                    s0 = ci * CH
                    ot_tiles = []
                    for h in range(2):
                        qt = [qt0, qt1][h]
                        n_kt = 4 * (ci + 1)
                        ot_ps = psOT.tile([128, CH], f32, name="ot_ps", tag="ot")
                        den_ps = psDEN.tile([1, CH], f32, name="den_ps", tag="den")
                        for kt in range(n_kt):
                            # diagonal tiles only contribute to columns >= off
                            ridx = kt - 4 * ci
                            off = max(ridx, 0) * 128
                            w = CH - off
                            st_ps = psST.tile([128, CH], f32, name="st_ps", tag="st")
                            nc.tensor.matmul(
                                st_ps[:, off:CH],
                                ktt[:, kt * 128:(kt + 1) * 128],
                                qt[:, s0 + off:s0 + CH],
                                start=True, stop=True,
                            )
                            p_sb = bp.tile([128, CH], DT, name="p_sb", tag="p")
                            nc.scalar.activation(
                                p_sb[:, off:CH], st_ps[:, off:CH], EXP, scale=SCALE
                            )
                            if ridx >= 0:
                                nc.vector.tensor_mul(
                                    p_sb[:, off:CH], p_sb[:, off:CH],
                                    mask_sb[:, ridx * CH + off:(ridx + 1) * CH],
                                )
                            nc.tensor.matmul(
                                ot_ps[:, off:CH],
                                vsb[:, kt * 128:(kt + 1) * 128],
                                p_sb[:, off:CH],
                                start=(kt == 0), stop=(kt == n_kt - 1),
                            )
                            nc.tensor.matmul(
                                den_ps[:, off:CH],
                                ones_sb[:, 0:1],
                                p_sb[:, off:CH],
                                start=(kt == 0), stop=(kt == n_kt - 1),
                            )
                        # normalization: den -> SBUF -> fp32 broadcast over
                        # partitions -> reciprocal -> multiply
                        den_sb = brp.tile([1, CH], f32, name="den_sb", tag="den_sb")
                        nc.scalar.copy(den_sb[:], den_ps[:])
                        bc_ps = psST.tile([128, CH], f32, name="bc_ps", tag="st")
                        nc.tensor.matmul(
                            bc_ps[:], ones32_sb[:], den_sb[:],
                            start=True, stop=True,
                        )
                        recip = brp.tile([128, CH], f32, name="recip", tag="recip")
                        nc.vector.reciprocal(recip[:], bc_ps[:])
                        ot_sb = bo.tile([128, CH], DT, name="ot_sb", tag=f"ot{h}")
                        nc.vector.tensor_mul(ot_sb[:], ot_ps[:], recip[:])
                        ot_tiles.append(ot_sb)

                    # Stage C: o_proj for this chunk's 4 row-tiles
                    for st_i in range(4):
                        row = (ci * 4 + st_i) * 128
                        for hc in range(4):
                            ops = psO.tile([128, 512], f32, name="ops", tag="ops")
                            nc.tensor.matmul(
                                ops[:],
                                ot_tiles[0][:, st_i * 128:(st_i + 1) * 128],
                                wo_sb0[:, hc * 512:(hc + 1) * 512],
                                start=True, stop=False,
                            )
                            nc.tensor.matmul(
                                ops[:],
                                ot_tiles[1][:, st_i * 128:(st_i + 1) * 128],
                                wo_sb1[:, hc * 512:(hc + 1) * 512],
                                start=False, stop=True,
                            )
                            o_sb = co.tile([128, 512], f32, name="o_sb", tag="o_sb")
                            if (st_i + hc) % 2 == 0:
                                nc.scalar.copy(o_sb[:], ops[:])
                            else:
                                nc.vector.tensor_copy(o_sb[:], ops[:])
                            nc.sync.dma_start(
                                out[row:row + 128, hc * 512:(hc + 1) * 512],
                                o_sb[:],
                            )

    nc.finalize()
    return nc


def _host_prep(hidden_states, cos, sin, position_ids, wq, wk, wv, wo):
    """Build the 8 per-core input maps."""
    if DT_NAME == "bf16":
        import ml_dtypes
        np_dt = ml_dtypes.bfloat16
    else:
        np_dt = np.float32
    tdt = np.float32

    hidden = np.asarray(hidden_states, dtype=np.float32)[0]        # [S, HID]
    hT = np.ascontiguousarray(hidden.T).astype(np_dt)              # [HID, S]
    pos = np.asarray(position_ids)[0].astype(np.int64)             # [S]
    cos_np = np.asarray(cos, dtype=np.float32)[pos]                # [S, 64]
    sin_np = np.asarray(sin, dtype=np.float32)[pos]
    cos_full = np.concatenate([cos_np, cos_np], axis=1)            # [S, 128]
    sin_full = np.concatenate([sin_np, sin_np], axis=1)
    cosT = np.ascontiguousarray(cos_full.T).astype(np_dt)          # [128, S]
    sinTs = np.ascontiguousarray(sin_full.T)
    sinTs[0:64, :] *= -1.0                                         # sign fold
    sinTs = sinTs.astype(np_dt)

    # multiplicative causal masks for the 4 diagonal tile offsets
    kk = np.arange(128)[:, None]
    jj = np.arange(CH)[None, :]
    mask4 = np.concatenate(
        [(kk + ridx * 128 <= jj).astype(np_dt) for ridx in range(4)], axis=1
    )                                                              # [128, 2048]
    ident = np.eye(128, dtype=tdt)
    onesm = np.ones((128, 128), dtype=np_dt)
    ones32 = np.ones((1, 128), dtype=np.float32)

    wq_np = np.asarray(wq, dtype=np.float32)
    wk_np = np.asarray(wk, dtype=np.float32)
    wv_np = np.asarray(wv, dtype=np.float32)
    wo_np = np.asarray(wo, dtype=np.float32)

    in_maps = []
    for c in range(NCORES):
        h0 = 2 * c
        g = c // 2
        wcat = np.ascontiguousarray(np.concatenate(
            [
                wq_np[:, h0 * D:(h0 + 1) * D],
                wq_np[:, (h0 + 1) * D:(h0 + 2) * D],
                wk_np[:, g * D:(g + 1) * D],
                wv_np[:, g * D:(g + 1) * D],
            ],
            axis=1,
        )).astype(np_dt)                                           # [HID, 512]
        wo2 = np.ascontiguousarray(
            wo_np[h0 * D:(h0 + 2) * D, :]
        ).astype(np_dt)                                            # [256, HID]
        in_maps.append({
            "hT": hT,
            "wcat": wcat,
            "wo2": wo2,
            "cosT": cosT,
            "sinTs": sinTs,
            "mask4": mask4,
            "ident": ident,
            "onesm": onesm,
            "ones32": ones32,
        })
    return in_maps


_NC_CACHE = [None]


def _run(inputs, trace=False, tmpdir=None):
    from concourse import bass_utils

    in_maps = _host_prep(
        inputs["hidden_states"], inputs["cos"], inputs["sin"],
        inputs["position_ids"], inputs["wq"], inputs["wk"], inputs["wv"],
        inputs["wo"],
    )
    if _NC_CACHE[0] is None:
        _NC_CACHE[0] = _build_nc()
    nc = _NC_CACHE[0]
    res = bass_utils.run_bass_kernel_spmd(
        nc, in_maps, core_ids=list(range(NCORES)), trace=trace, tmpdir=tmpdir,
    )
    acc = res.results[0]["out"].astype(np.float32)
    for c in range(1, NCORES):
        acc = acc + res.results[c]["out"]
    return acc.reshape(1, S, HID), res


def kernel(**inputs):
    out, _ = _run(inputs, trace=False)
    return out
